# revision 2
# baseline (speedup 1.0000x reference)
"""Trainium2 Bass kernel for nn_AbsDiagNetGated.

Computation (reference):
    g    = relu(einsum('tbi,gi->tbg', X, W_ih))      # [T,B,G]
    proj = einsum('tbg,hg->tbh', g, W_cell)          # [T,B,H]
    scan: h_t = |proj_t + HH*h_{t-1}|, h_0 = 0       # elementwise over [B,H]
    out  = h_T @ W_ho.T + b_ho                       # [B,O]

Strategy: data-parallel over batch B across 8 cores (16 batch rows each),
weights replicated.  The two big GEMMs run fused per 32-timestep block with
the contraction dim on partitions (X pre-transposed host-side), outputs kept
transposed ([feature, row]) so the block scan state lives in a single
[128, h_hi*16+b] tile and each scan step is two DVE instructions:
abs via int32 AND, then add of proj_t.  Matmuls use float32r (full PE rate).
"""

import numpy as np

import concourse.bacc as bacc
import concourse.mybir as mybir
import concourse.tile as tile
from concourse.bass_utils import run_bass_kernel_spmd

T, B, I = 512, 128, 512
G, H, O = 1024, 1024, 512

N_CORES = 8
BS = B // N_CORES          # 16 batch rows per core
TBLK = 32                  # timesteps per block
NBLK = T // TBLK           # 16 blocks
R = TBLK * BS              # 512 rows (moving dim) per block

F32 = mybir.dt.float32
F32R = mybir.dt.float32r
I32 = mybir.dt.int32
ALU = mybir.AluOpType
ACTF = mybir.ActivationFunctionType

KI = I // 128              # 4  k-tiles for GEMM 1
NG = G // 128              # 8  m-tiles (G) for GEMM 1 == k-tiles for GEMM 2
NH = H // 128              # 8  m-tiles (H) for GEMM 2
NO = O // 128              # 4  m-tiles (O) for the final GEMM


def _r(ap):
    return ap.bitcast(F32R)


def _build(hh_is_one: bool):
    nc = bacc.Bacc("TRN2", target_bir_lowering=False, debug=False)

    xt_d = nc.dram_tensor("xt", [I, T, BS], F32R, kind="ExternalInput")
    wih_d = nc.dram_tensor("wih_t", [I, G], F32R, kind="ExternalInput")
    wcell_d = nc.dram_tensor("wcell_t", [G, H], F32R, kind="ExternalInput")
    who_d = nc.dram_tensor("who_t", [H, O], F32, kind="ExternalInput")
    bho_d = nc.dram_tensor("bho", [O, 1], F32, kind="ExternalInput")
    hh_d = None
    if not hh_is_one:
        hh_d = nc.dram_tensor("hh_rep", [128, 128], F32, kind="ExternalInput")
    out_d = nc.dram_tensor("out_t", [O, BS], F32, kind="ExternalOutput")

    xt_r = xt_d.ap().rearrange("(ki p) t b -> ki p (t b)", p=128)
    wih_r = wih_d.ap().rearrange("(ki p) g -> ki p g", p=128)
    wcell_r = wcell_d.ap().rearrange("(kg p) h -> kg p h", p=128)
    who_r = who_d.ap().rearrange("(kh p) o -> kh p o", p=128)
    bho_r = bho_d.ap().rearrange("(ot p) one -> ot p one", p=128)
    out_r = out_d.ap().rearrange("(ot p) b -> p ot b", p=128)

    with tile.TileContext(nc) as tc:
        with (
            tc.tile_pool(name="consts", bufs=1) as cpool,
            tc.tile_pool(name="xt_pool", bufs=3) as xpool,
            tc.tile_pool(name="g_pool", bufs=2) as gpool,
            tc.tile_pool(name="proj_pool", bufs=2) as ppool,
            tc.tile_pool(name="state", bufs=1) as spool,
            tc.tile_pool(name="psum1", bufs=2, space="PSUM") as ps1pool,
            tc.tile_pool(name="psum2", bufs=3, space="PSUM") as ps2pool,
            tc.tile_pool(name="psum3", bufs=2, space="PSUM") as ps3pool,
        ):
            wih = []
            for ki in range(KI):
                w = cpool.tile([128, G], F32R, name=f"wih_{ki}", tag=f"wih_{ki}")
                nc.sync.dma_start(out=w[:], in_=wih_r[ki])
                wih.append(w)
            wcell = []
            for kg in range(NG):
                w = cpool.tile([128, H], F32R, name=f"wcell_{kg}", tag=f"wcell_{kg}")
                nc.sync.dma_start(out=w[:], in_=wcell_r[kg])
                wcell.append(w)
            who = []
            for kh in range(NH):
                w = cpool.tile([128, O], F32, name=f"who_{kh}", tag=f"who_{kh}")
                nc.sync.dma_start(out=w[:], in_=who_r[kh])
                who.append(w)
            bias = cpool.tile([128, NO], F32, name="bias", tag="bias")
            for ot in range(NO):
                nc.sync.dma_start(out=bias[:, ot : ot + 1], in_=bho_r[ot])
            hh = None
            if hh_d is not None:
                hh = cpool.tile([128, 128], F32, name="hh", tag="hh")
                nc.sync.dma_start(out=hh[:], in_=hh_d.ap())

            # scan state: s = pre-abs state, a = |s| (both [128, h_hi*16+b])
            s = spool.tile([128, 128], F32, name="s", tag="s")
            a = spool.tile([128, 128], F32, name="a", tag="a")
            nc.vector.memset(s[:], 0.0)

            for blk in range(NBLK):
                xt = []
                for ki in range(KI):
                    x = xpool.tile([128, R], F32R, name=f"xt_{ki}", tag=f"xt_{ki}")
                    nc.sync.dma_start(
                        out=x[:], in_=xt_r[ki, :, blk * R : (blk + 1) * R]
                    )
                    xt.append(x)

                g = []
                for gt in range(NG):
                    ps1 = ps1pool.tile([128, R], F32, name="ps1", tag="ps1")
                    for ki in range(KI):
                        nc.tensor.matmul(
                            ps1[:],
                            wih[ki][:, gt * 128 : (gt + 1) * 128],
                            xt[ki][:],
                            start=(ki == 0),
                            stop=(ki == KI - 1),
                        )
                    gg = gpool.tile([128, R], F32R, name=f"g_{gt}", tag=f"g_{gt}")
                    nc.scalar.activation(gg[:], ps1[:], ACTF.Relu)
                    g.append(gg)

                proj = ppool.tile([128, TBLK * 128], F32, name="proj", tag="proj")
                proj_v = proj.rearrange("p (t x) -> p t x", x=128)
                for ht in range(NH):
                    ps2 = ps2pool.tile([128, R], F32, name="ps2", tag="ps2")
                    for kg in range(NG):
                        nc.tensor.matmul(
                            ps2[:],
                            wcell[kg][:, ht * 128 : (ht + 1) * 128],
                            g[kg][:],
                            start=(kg == 0),
                            stop=(kg == NG - 1),
                        )
                    # [p, (t b)] -> proj[p, t, ht*16 + b]
                    nc.scalar.activation(
                        proj_v[:, :, ht * BS : (ht + 1) * BS],
                        ps2.rearrange("p (t b) -> p t b", b=BS),
                        ACTF.Copy,
                    )

                for tl in range(TBLK):
                    # a = |s| (bitwise clear of the sign bit)
                    nc.vector.tensor_scalar(
                        a.bitcast(I32)[:],
                        s.bitcast(I32)[:],
                        0x7FFFFFFF,
                        None,
                        ALU.bitwise_and,
                    )
                    p_t = proj[:, tl * 128 : (tl + 1) * 128]
                    if hh is None:
                        # s' = a + p_t
                        nc.vector.tensor_tensor(s[:], a[:], p_t, ALU.add)
                    else:
                        # s' = a*hh + p_t
                        nc.vector.tensor_tensor(a[:], a[:], hh[:], ALU.mult)
                        nc.vector.tensor_tensor(s[:], a[:], p_t, ALU.add)

            # h_T = |s|
            nc.vector.tensor_scalar(
                a.bitcast(I32)[:], s.bitcast(I32)[:], 0x7FFFFFFF, None, ALU.bitwise_and
            )

            out_sb = spool.tile([128, NO * BS], F32, name="out_sb", tag="out_sb")
            for ot in range(NO):
                ps3 = ps3pool.tile([128, BS], F32, name="ps3", tag="ps3")
                for kh in range(NH):
                    nc.tensor.matmul(
                        ps3[:],
                        who[kh][:, ot * 128 : (ot + 1) * 128],
                        a[:, kh * BS : (kh + 1) * BS],
                        start=(kh == 0),
                        stop=(kh == NH - 1),
                    )
                nc.scalar.activation(
                    out_sb[:, ot * BS : (ot + 1) * BS],
                    ps3[:],
                    ACTF.Identity,
                    bias=bias[:, ot : ot + 1],
                )
            nc.sync.dma_start(
                out=out_r, in_=out_sb.rearrange("p (ot b) -> p ot b", b=BS)
            )

    nc.compile()
    return nc


_BUILD_CACHE: dict = {}


def _get_nc(hh_is_one: bool):
    if hh_is_one not in _BUILD_CACHE:
        _BUILD_CACHE[hh_is_one] = _build(hh_is_one)
    return _BUILD_CACHE[hh_is_one]


def _make_in_maps(X, W_ih, W_cell, HH, W_ho, b_ho, hh_is_one):
    xt = np.ascontiguousarray(np.transpose(np.asarray(X, np.float32), (2, 0, 1)))
    wih_t = np.ascontiguousarray(np.asarray(W_ih, np.float32).T)
    wcell_t = np.ascontiguousarray(np.asarray(W_cell, np.float32).T)
    who_t = np.ascontiguousarray(np.asarray(W_ho, np.float32).T)
    bho = np.ascontiguousarray(np.asarray(b_ho, np.float32).reshape(O, 1))

    in_maps = []
    for c in range(N_CORES):
        m = {
            "xt": np.ascontiguousarray(xt[:, :, c * BS : (c + 1) * BS]),
            "wih_t": wih_t,
            "wcell_t": wcell_t,
            "who_t": who_t,
            "bho": bho,
        }
        if not hh_is_one:
            # hh_rep[p, h_hi*16 + b] = HH[h_hi*128 + p]
            hh_rep = np.repeat(
                np.asarray(HH, np.float32).reshape(NH, 128).T, BS, axis=1
            )
            m["hh_rep"] = np.ascontiguousarray(hh_rep)
        in_maps.append(m)
    return in_maps


def kernel(X, W_ih, W_cell, HH, W_ho, b_ho):
    HH = np.asarray(HH, np.float32)
    hh_is_one = bool(np.all(HH == 1.0))
    nc = _get_nc(hh_is_one)
    in_maps = _make_in_maps(X, W_ih, W_cell, HH, W_ho, b_ho, hh_is_one)
    res = run_bass_kernel_spmd(nc, in_maps, core_ids=list(range(N_CORES)))
    out = np.empty((B, O), np.float32)
    for c in range(N_CORES):
        out[c * BS : (c + 1) * BS, :] = res.results[c]["out_t"].T
    return out
